# revision 1
# baseline (speedup 1.0000x reference)
"""Conv2D (VALID, 3x3, NCHW) via 1D Winograd F(2,3) along W, on 8 TRN2 cores.

Problem: x (32,128,56,56) f32, weight (256,128,3,3) f32, bias (256,) f32
         -> out (32,256,54,54) f32.

Strategy:
  - Data-parallel over batch: 4 images per core, no collectives.
  - 1D Winograd F(2,3) along W: for output col pair (2j, 2j+1),
      d0..d3 = x[.., 2j..2j+3]
      V0 = d0-d2, V1 = d1+d2, V2 = d2-d1, V3 = d1-d3      (device, DVE/GpSimd)
      Wq = G-transformed weights along kw (host)
      M_q = sum_kh sum_cin Wq[kh] * Vq(rows shifted by kh)  (PE, PSUM accum)
      y_even = M0+M1+M2+b ; y_odd = M1-M2-M3+b             (DVE combine)
    PE cols drop 1.5x vs direct conv: 288 matmuls x 486 cols per core.
  - Host uploads x as 4 shifted/parity planes (A,B,C,D) in bf16 so the
    V-transform runs as aligned step-1 tensor_tensor ops.
  - Output written bf16 (halves store traffic); host upcasts to f32.
"""

import numpy as np
import ml_dtypes

import concourse.bass as bass
import concourse.mybir as mybir
from concourse import bacc
import concourse.tile as tile
from concourse.tile import add_dep_helper
from concourse.bass_utils import run_bass_kernel_spmd

N, CIN, H, W = 32, 128, 56, 56
COUT, KH, KW = 256, 3, 3
HO, WO = H - KH + 1, W - KW + 1  # 54, 54
NCORES = 8
NPER = N // NCORES   # 4
CTILES = COUT // 128  # 2
JT = WO // 2          # 27 tiles along W
RCH = 18              # output rows per chunk
NCH = HO // RCH       # 3 chunks per (img, ctile)
NPIX = RCH * JT       # 486 <= 512 (one fp32 PSUM bank)
WCOLS = CTILES * 4 * KH * 128  # 3072

BF16 = mybir.dt.bfloat16
F32 = mybir.dt.float32
ADD = mybir.AluOpType.add
SUB = mybir.AluOpType.subtract


def build_nc() -> bass.Bass:
    nc = bacc.Bacc(None)
    x_h = nc.dram_tensor("x", [NPER, CIN, 4, H * JT], BF16, kind="ExternalInput")
    w_h = nc.dram_tensor("w", [CIN, WCOLS], BF16, kind="ExternalInput")
    b_h = nc.dram_tensor("b", [COUT, 1], F32, kind="ExternalInput")
    o_h = nc.dram_tensor("out", [NPER, COUT, 2, HO * JT], BF16, kind="ExternalOutput")

    with tile.TileContext(nc) as tc:
        with (
            tc.tile_pool(name="wpool", bufs=1) as wpool,
            tc.tile_pool(name="bpool", bufs=1) as bpool,
            tc.tile_pool(name="xpool", bufs=4) as xpool,
            tc.tile_pool(name="vpool", bufs=4) as vpool,
            tc.tile_pool(name="tpool", bufs=6) as tpool,
            tc.tile_pool(name="opool", bufs=4) as opool,
            tc.tile_pool(name="psum", bufs=8, space="PSUM") as psum_pool,
        ):
            # PE warmup for HAM un-throttle during the input-DMA window.
            wu = wpool.tile([CIN, 64], BF16)
            nc.gpsimd.memset(wu[:], 0)
            wupt = psum_pool.tile([32, 64], F32, tag="pt")
            warmups = []
            for _ in range(96):
                warmups.append(
                    nc.tensor.matmul(wupt[:], wu[:, :32], wu[:, :64], start=True, stop=True)
                )

            # Input DMAs. Critical: ct0 weights (sync) + img0 plane rows 0-19
            # (scalar). Everything else deferred behind matmul anchors.
            wt = wpool.tile([CIN, WCOLS], BF16)
            nc.sync.dma_start(out=wt[:, : WCOLS // 2], in_=w_h[:, : WCOLS // 2])

            xts = []
            for n in range(NPER):
                xts.append(xpool.tile([CIN, 4, H * JT], BF16, tag="xt", name=f"xt{n}"))
            x0a = [
                nc.scalar.dma_start(
                    out=xts[0][:, q, 0 : 20 * JT], in_=x_h[0, :, q, 0 : 20 * JT]
                )
                for q in range(4)
            ]
            x0b = [
                nc.scalar.dma_start(
                    out=xts[0][:, q, 20 * JT : 38 * JT], in_=x_h[0, :, q, 20 * JT : 38 * JT]
                )
                for q in range(4)
            ]
            x0c = [
                nc.scalar.dma_start(
                    out=xts[0][:, q, 38 * JT : 56 * JT], in_=x_h[0, :, q, 38 * JT : 56 * JT]
                )
                for q in range(4)
            ]
            bias_t = bpool.tile([COUT // CTILES, CTILES], F32)
            nc.sync.dma_start(
                out=bias_t[:], in_=b_h.rearrange("(c p) o -> p (c o)", p=128)
            )
            w2 = nc.sync.dma_start(out=wt[:, WCOLS // 2 :], in_=w_h[:, WCOLS // 2 :])
            xdmas = [None]
            for n in range(1, NPER):
                xdmas.append(nc.sync.dma_start(out=xts[n][:], in_=x_h[n]))

            for d in x0b:
                add_dep_helper(d.ins, warmups[40].ins, reason="defer x0b")
            add_dep_helper(w2.ins, warmups[55].ins, reason="defer w2")

            # V transform: V0=A-C, V1=B+C, V2=C-B, V3=B-D  (planes 0..3)
            vts = []
            for n in range(NPER):
                vts.append(vpool.tile([CIN, 4, H * JT], BF16, tag="vt", name=f"vt{n}"))

            def vtrans(eng, n, r0, r1):
                xt, vt = xts[n], vts[n]
                a, b = r0 * JT, r1 * JT
                eng.tensor_sub(vt[:, 0, a:b], xt[:, 0, a:b], xt[:, 2, a:b])
                eng.tensor_add(vt[:, 1, a:b], xt[:, 1, a:b], xt[:, 2, a:b])
                eng.tensor_sub(vt[:, 2, a:b], xt[:, 2, a:b], xt[:, 1, a:b])
                eng.tensor_sub(vt[:, 3, a:b], xt[:, 1, a:b], xt[:, 3, a:b])

            # All V transforms on DVE: flat even-length step-1 bf16 APs hit
            # the 2x packed mode (~0.9us/plane); GpSimd measured 3.2-3.4us.
            vtrans(nc.vector, 0, 0, 20)
            vtrans(nc.vector, 0, 20, 38)
            vtrans(nc.vector, 0, 38, 56)
            for n in range(1, NPER):
                vtrans(nc.vector, n, 0, 56)

            add_dep_helper(xdmas[1].ins, warmups[70].ins, reason="defer x1")
            deferred = {0: list(x0c), 6: [xdmas[2]], 60: [xdmas[3]]}
            mm_idx = 0

            for n in range(NPER):
                for c in range(CTILES):
                    for hc in range(NCH):
                        h0 = hc * RCH
                        pts = [
                            psum_pool.tile([128, NPIX], F32, tag="pt", name=f"pt{n}_{c}_{hc}_{q}")
                            for q in range(4)
                        ]
                        for q in range(4):
                            for kh in range(KH):
                                off = ((c * 4 + q) * KH + kh) * 128
                                mm = nc.tensor.matmul(
                                    pts[q][:],
                                    wt[:, off : off + 128],
                                    vts[n][:, q, (h0 + kh) * JT : (h0 + kh + RCH) * JT],
                                    start=(kh == 0),
                                    stop=(kh == KH - 1),
                                )
                                for dma in deferred.get(mm_idx, ()):
                                    add_dep_helper(dma.ins, mm.ins, reason="defer DMA")
                                mm_idx += 1
                        # PSUM+PSUM tensor_tensor is illegal (NCC_IBVF027):
                        # ACT stages M1/M2 to SBUF (frees those banks early,
                        # overlapping the tail of the chunk's matmuls), DVE
                        # does the adds with at most one PSUM operand per op
                        # and two pure-SBUF STTs (cheapest DVE form).
                        ot = opool.tile([128, 2, NPIX], BF16, tag="ot")
                        t0 = tpool.tile([128, NPIX], F32, tag="t0")
                        w1 = tpool.tile([128, NPIX], F32, tag="w1")
                        m1s = tpool.tile([128, NPIX], F32, tag="m1s")
                        m2s = tpool.tile([128, NPIX], F32, tag="m2s")
                        bsl = bias_t[:, c : c + 1]
                        nc.scalar.copy(m1s[:], pts[1][:])
                        nc.scalar.copy(m2s[:], pts[2][:])
                        nc.vector.tensor_add(t0[:], pts[0][:], m1s[:])
                        nc.vector.scalar_tensor_tensor(
                            ot[:, 0], t0[:], bsl, m2s[:], ADD, ADD
                        )
                        nc.vector.scalar_tensor_tensor(
                            w1[:], m1s[:], bsl, m2s[:], ADD, SUB
                        )
                        nc.vector.tensor_sub(ot[:, 1], w1[:], pts[3][:])
                        co = c * 128
                        last = n == NPER - 1 and c == CTILES - 1 and hc == NCH - 1
                        ha, hb = h0 * JT, (h0 + RCH) * JT
                        # last few chunks alternate rings so the final output
                        # DMAs drain in parallel instead of serially on ACT
                        chunk_id = (n * CTILES + c) * NCH + hc
                        ring = nc.sync if chunk_id >= 20 and chunk_id % 2 == 0 else nc.scalar
                        if not last:
                            ring.dma_start(
                                out=o_h[n, co : co + 128, :, ha:hb], in_=ot[:]
                            )
                        else:
                            nc.scalar.dma_start(
                                out=o_h[n, co : co + 128, 0, ha:hb], in_=ot[:, 0]
                            )
                            nc.sync.dma_start(
                                out=o_h[n, co : co + 128, 1, ha:hb], in_=ot[:, 1]
                            )
    nc.finalize()
    return nc


_NC_CACHE = None


def _get_nc():
    global _NC_CACHE
    if _NC_CACHE is None:
        _NC_CACHE = build_nc()
    return _NC_CACHE


def _prep_in_maps(x, weight, bias):
    bf16 = ml_dtypes.bfloat16
    w = weight.astype(np.float32)
    g0, g1, g2 = w[:, :, :, 0], w[:, :, :, 1], w[:, :, :, 2]  # [COUT, CIN, KH]
    Wq = np.stack([g0, (g0 + g1 + g2) * 0.5, (g0 - g1 + g2) * 0.5, g2], axis=0)
    # layout [CIN, ct, q, kh, m] -> [CIN, 3072]
    Wt = np.zeros((CIN, CTILES, 4, KH, 128), np.float32)
    for ct in range(CTILES):
        Wt[:, ct] = Wq[:, ct * 128 : (ct + 1) * 128].transpose(2, 0, 3, 1)
    w_t = np.ascontiguousarray(Wt.reshape(CIN, WCOLS)).astype(bf16)
    b_t = np.ascontiguousarray(bias.astype(np.float32).reshape(COUT, 1))
    # x planes: A=x[0::2](27), B=x[1::2](27), C=x[2::2](27), D=x[3::2](27)
    P = np.stack(
        [x[:, :, :, 0:54:2], x[:, :, :, 1:55:2], x[:, :, :, 2:56:2], x[:, :, :, 3:56:2]],
        axis=2,
    ).reshape(N, CIN, 4, H * JT)  # [N, CIN, 4, H*27]
    in_maps = []
    for i in range(NCORES):
        xs = np.ascontiguousarray(P[i * NPER : (i + 1) * NPER]).astype(bf16)
        in_maps.append({"x": xs, "w": w_t, "b": b_t})
    return in_maps


def run(x, weight, bias, trace=False):
    nc = _get_nc()
    in_maps = _prep_in_maps(x, weight, bias)
    res = run_bass_kernel_spmd(nc, in_maps, core_ids=list(range(NCORES)), trace=trace)
    o = np.concatenate([r["out"] for r in res.results], axis=0).reshape(
        N, COUT, 2, HO, JT
    )
    out = np.empty((N, COUT, HO, WO), np.float32)
    out[:, :, :, 0::2] = o[:, :, 0].astype(np.float32)
    out[:, :, :, 1::2] = o[:, :, 1].astype(np.float32)
    return out, res


def kernel(x: np.ndarray, weight: np.ndarray, bias: np.ndarray) -> np.ndarray:
    out, _ = run(x, weight, bias, trace=False)
    return out.astype(np.float32)

